# revision 38
# baseline (speedup 1.0000x reference)
"""Trainium2 Bass kernel for a 16-step neural cellular automaton (BasicNCA).

Reference semantics (per step):
    c   = conv3x3(x, k, SAME)                    # 1 channel
    g   = exp(-(c-1)^2)
    h   = relu(g*w1 + b1); o = sigmoid(h@w2)     # pointwise 1->10->1 MLP
    x  += o - 0.5
Output: all 17 states stacked, [17, 16, 1, 512, 512] f32.

Kernel architecture (per core: 2 images, pure data parallel over 8 cores):
 * The pointwise chain collapses to Delta(c) = A + Q*exp(-(alpha*(c-1))^2)
   (fit on host from the actual weights; the Gaussian bump is exact up to
   the near-linearity of the tiny MLP in g).  The Gaussian is ONE ScalarE
   pass: Derivative_Erf(alpha*c + bias) = (2/sqrt(pi))*exp(-.) -> fp8 out.
 * SBUF state x excludes the s*A drift (added back on host); the conv of
   the A-field is folded into a per-step per-partition ACT bias table plus
   two constant column-edge rows in the halo matmul.
 * Conv state c lives in PSUM (all 8 banks, 4 pairs of [128,1024]) and is
   updated incrementally: c += conv3x3(q_s * G8).  The scale q_s and the
   3x3 taps are folded into fp8 DoubleRow banded matmuls at 0.5 cycles/
   row: 2 per 128-row block (column taps {j0,j2}_hi via i-stride 2 from
   base-1, {j1_hi,j1_lo} via i-stride 0).  The j1 hi+lo split kills the
   dominant fp8 weight-quantization bias; SAME column padding is
   automatic via interleaved zero pad columns in the dl8 layout.
 * Row-halo exchange between the 4 row-tile pairs: 4 tiny SBUF->SBUF row
   copies per boundary (unshifted + pre-shifted by one column, split
   over the sync and gpsimd DGE rings) into an h8 tensor mirroring the
   dl8 layout on 6 partitions (above, above+1, below, below+1, and two
   static column-edge const masks).  ONE DoubleRow matmul per block
   (i-stride 2 from base-1) then applies all six halo/const terms.
 * x update is one VectorE scalar_tensor_tensor per pair:
   x_new = (q_s * G8) + x_cur, fp8 in / f32 out.
 * Output states are emitted one step late in 512KB per-pair chunks so
   the big output transfers never block the latency-critical halo DMAs
   on the shared DMA-engine pool; ACT visit order [1,0,2,3] lets the
   single-neighbour T/B pairs stop their PSUM groups early.
 * Only steady-state DRAM traffic: the mandatory 2 MB/core/step output.
"""

import math

import numpy as np
import ml_dtypes

P = 128            # partitions
W = 512            # image width
TPI = 4            # row-tiles per image
NIMG = 2           # images per core
NCORES = 8
NPAIR = 4          # PSUM pairs: pair t = (img0 tile t, img1 tile t)
BW = W + 1         # padded block stride in dl8
DL = 1 + NIMG * TPI * BW     # dl8 free size (lead pad + 8 blocks)
XF = NIMG * TPI * W          # x free size (t, b, c)
PF = NIMG * W                # PSUM pair free size
HF = NPAIR * PF              # h8 per-slot free size

F8NP = ml_dtypes.float8_e4m3

# Fit of (A, Q, alpha) for the setup_inputs() weights; refit on mismatch.
_DEFAULT_PARAMS = (0.02216485, 0.17021647, 1.05195449)

_NC_CACHE = {}
LAST_RESULTS = None


# --------------------------------------------------------------------------
# Host-side model fitting
# --------------------------------------------------------------------------

def _delta_exact(c, w1, b1, w2):
    g = np.exp(-(c - 1.0) ** 2)
    z = g[..., None] * w1.reshape(-1) + b1.reshape(-1)
    pv = (np.maximum(z, 0.0) * w2.reshape(-1)).sum(-1)
    return 1.0 / (1.0 + np.exp(-pv)) - 0.5


def _model(p, c):
    return p[0] + p[1] * np.exp(-np.square(p[2] * (c - 1.0)))


def _get_params(w1, b1, w2):
    grid = np.linspace(-26.0, 26.0, 40001)
    target = _delta_exact(grid, w1, b1, w2)
    p0 = np.array(_DEFAULT_PARAMS)
    if float(np.abs(_model(p0, grid) - target).max()) < 4e-3:
        return tuple(p0)
    try:
        from scipy.optimize import least_squares
        A0 = float(target[0])
        Q0 = float(target[grid.searchsorted(1.0)]) - A0
        best = (np.inf, p0)
        for al0 in (0.5, 0.8, 1.05, 1.4, 2.0):
            try:
                sol = least_squares(lambda p: _model(p, grid) - target,
                                    [A0, Q0, al0], max_nfev=20000)
                e = float(np.abs(_model(sol.x, grid) - target).max())
                if e < best[0]:
                    best = (e, sol.x)
            except Exception:
                pass
        return tuple(float(v) for v in best[1])
    except Exception:
        return tuple(p0)


# --------------------------------------------------------------------------
# Bass program
# --------------------------------------------------------------------------

def _build_nc(kk, params, steps):
    from concourse import bacc, mybir, tile
    from concourse.bass_types import AP

    f32 = mybir.dt.float32
    f16 = mybir.dt.float16
    f8 = mybir.dt.float8e4
    AF = mybir.ActivationFunctionType
    OP = mybir.AluOpType
    DR = mybir.MatmulPerfMode.DoubleRow

    A_, Q_, al_ = [float(v) for v in params]
    q_s = Q_ * math.sqrt(math.pi) / 2.0
    kk = np.asarray(kk, np.float64).reshape(3, 3)
    kq = kk * q_s

    nc = bacc.Bacc("TRN2", target_bir_lowering=False, debug=False,
                   num_devices=NCORES)
    x_in = nc.dram_tensor("x", [NIMG, W, W], f32, kind="ExternalInput")
    out = nc.dram_tensor("out", [steps + 1, NIMG, W, W], f32,
                         kind="ExternalOutput")

    # ---- host-built constants --------------------------------------------
    def banded(ktaps):
        # band[q, p] = ktaps[1 + (q - p)] for |q-p| <= 1 (row taps)
        m = np.zeros((P, P), np.float64)
        for dr in (-1, 0, 1):
            for p in range(P):
                q = p + dr
                if 0 <= q < P:
                    m[q, p] = ktaps[1 + dr]
        return m

    # fp8 hi+lo of the three column-tap banded matrices (with q_s folded)
    Bh, Bl = [], []
    for j in range(3):
        b = banded(kq[:, j])
        hi = b.astype(F8NP)
        lo = (b - hi.astype(np.float64)).astype(F8NP)
        Bh.append(hi)
        Bl.append(lo)
    # DoubleRow lhsT tensors [K, i, M] stored [128, 256] fp8 (i-major):
    # w1: (B0_hi, B2_hi) i-stride 2; w2: (B1_hi, B1_lo) i-stride 0
    wmm_h = [
        np.concatenate([Bh[0], Bh[2]], axis=1),
        np.concatenate([Bh[1], Bl[1]], axis=1),
    ]
    wmm_t = [nc.inline_tensor(w.astype(F8NP), name=f"wmm{i}")
             for i, w in enumerate(wmm_h)]

    # fp16 init banded matrices (exact k, no q_s)
    k16 = kk.astype(np.float16).astype(np.float64)
    wi_h = [banded(k16[:, j]).astype(np.float16) for j in range(3)]
    wi_t = [nc.inline_tensor(wi_h[j], name=f"wi{j}") for j in range(3)]

    # halo weights, per variant (T/M/B).  h8 mirrors dl8's padded layout
    # with 6 partitions: 0=above, 1=above shifted +1, 2=below, 3=below
    # shifted +1, 4=const-col0 mask (1 at base+1), 5=const-col511 mask
    # (1 at base+512).  ONE DoubleRow matmul per block: i-stride 2 from
    # base-1 (reads offsets n-1 and n+1):
    #   p0: (j0, j2); p1: (0, j1 via shift); p2: (j0, j2); p3: (0, j1);
    #   p4: (0, const0 w);  p5: (0, const511 w)
    kq8 = kq.astype(F8NP).astype(np.float64)

    def whm_variant(v):
        m = np.zeros((6, 2, P), np.float64)
        if v != "T":
            m[0, 0, 0] = kq8[0, 0]
            m[0, 1, 0] = kq8[0, 2]
            m[1, 1, 0] = kq8[0, 1]
        if v != "B":
            m[2, 0, P - 1] = kq8[2, 0]
            m[2, 1, P - 1] = kq8[2, 2]
            m[3, 1, P - 1] = kq8[2, 1]
        # column-edge const corrections (exact k, scaled by A each step)
        for p in range(P):
            rows = [i for i in range(3)
                    if not (v == "T" and p == 0 and i == 0)
                    and not (v == "B" and p == P - 1 and i == 2)]
            m[4, 1, p] = -A_ * sum(kk[i, 0] for i in rows)
            m[5, 1, p] = -A_ * sum(kk[i, 2] for i in rows)
        return m.reshape(6, 2 * P)

    whm_t = {v: nc.inline_tensor(whm_variant(v).astype(F8NP), name=f"whm{v}")
             for v in "TMB"}

    # init halo weights (fp16, exact k16, data rows only): [6, 128]
    def hwi_variant(v):
        m = np.zeros((6, P), np.float16)
        if v != "T":
            for j in range(3):
                m[j, 0] = k16[0, j]
        if v != "B":
            for j in range(3):
                m[3 + j, P - 1] = k16[2, j]
        return m

    whi_t = {v: nc.inline_tensor(hwi_variant(v), name=f"whi{v}")
             for v in "TMB"}

    # h8 const partitions 4/5 (shifted masks read at offset n+1):
    # const0 fires at n=0 -> 1 at base+1; const511 at n=511 -> base+512
    hconst = np.zeros((2, DL), np.float32)
    for blk in range(NIMG * TPI):
        base = 1 + blk * BW
        hconst[0, base + 1] = 1.0
        hconst[1, base + W] = 1.0
    hconst_t = nc.inline_tensor(hconst.astype(F8NP), name="hconst")

    # per-step per-variant ACT bias table [128, steps*3]: alpha*(s*F_v - 1)
    Sall = float(kk.sum())
    Stop = float(kk[1:, :].sum())     # row 0 of image: no row above
    Sbot = float(kk[:2, :].sum())     # last row: no row below
    bias_h = np.zeros((P, steps * 3), np.float64)
    for s in range(steps):
        for vi, v in enumerate("TMB"):
            col = np.full(P, A_ * Sall)
            if v == "T":
                col[0] = A_ * Stop
            if v == "B":
                col[P - 1] = A_ * Sbot
            bias_h[:, s * 3 + vi] = al_ * (s * col - 1.0)
    bias_t = nc.inline_tensor(bias_h.astype(np.float32), name="biastab")

    VAR = ["T", "M", "M", "B"]
    VIDX = {"T": 0, "M": 1, "B": 2}

    # ---- on-chip tensors -------------------------------------------------
    xb = [nc.alloc_sbuf_tensor(f"xs{i}", [P, XF], f32) for i in range(2)]
    dl8 = [nc.alloc_sbuf_tensor(f"dl8_{i}", [P, DL], f8)
           for i in range(2)]
    dl16 = nc.alloc_sbuf_tensor("dl16", [P, DL], f16)
    h8 = [nc.alloc_sbuf_tensor(f"h8_{i}", [6, DL], f8) for i in range(2)]
    h16i = nc.alloc_sbuf_tensor("h16i", [6, HF], f16)
    wmm = [nc.alloc_sbuf_tensor(f"wmm{i}s", [P, 2 * P], f8) for i in range(2)]
    wi = [nc.alloc_sbuf_tensor(f"wi{j}s", [P, P], f16) for j in range(3)]
    whm = {v: nc.alloc_sbuf_tensor(f"whm{v}s", [6, 2 * P], f8) for v in "TMB"}
    whi = {v: nc.alloc_sbuf_tensor(f"whi{v}s", [6, P], f16) for v in "TMB"}
    bias = nc.alloc_sbuf_tensor("bias", [P, steps * 3], f32)

    def blk_off(t, b):
        # dl8/dl16 data col offset of block (pair t, image b)
        return 1 + (2 * t + b) * BW

    with tile.TileContext(nc) as tc:
        with tc.tile_pool(name="psum", bufs=1, space="PSUM") as pp:
            cps = [pp.tile([P, PF], f32, tag=f"c{t}", name=f"c{t}")
                   for t in range(NPAIR)]

            # ---------------- init ----------------
            for i in range(3):
                if i < 2:
                    nc.sync.dma_start(out=wmm[i].ap(), in_=wmm_t[i].ap())
                nc.sync.dma_start(out=wi[i].ap(), in_=wi_t[i].ap())
            for v in "TMB":
                nc.sync.dma_start(out=whm[v].ap(), in_=whm_t[v].ap())
                nc.sync.dma_start(out=whi[v].ap(), in_=whi_t[v].ap())
            nc.sync.dma_start(out=bias.ap(), in_=bias_t.ap())
            for i in range(2):
                nc.vector.memset(h8[i].ap(), 0.0)
                nc.sync.dma_start(out=h8[i].ap()[4:6, :], in_=hconst_t.ap())
            nc.vector.memset(h16i.ap(), 0.0)
            # pad columns of dl8 / dl16 (stride BW, 9 of them)
            for i in range(2):
                nc.vector.memset(
                    AP(tensor=dl8[i].ap().tensor,
                       ap=[[DL, P], [BW, 9], [1, 1]], offset=0), 0.0)
            nc.vector.memset(
                AP(tensor=dl16.ap().tensor,
                   ap=[[DL, P], [BW, 9], [1, 1]], offset=0), 0.0)

            # load x0 into (t, b, c) layout (per image: 3-dim APs)
            xin_v = x_in.rearrange("b (t p) c -> p b t c", p=P)
            for b in range(NIMG):
                nc.sync.dma_start(
                    out=AP(tensor=xb[0].ap().tensor,
                           ap=[[XF, P], [NIMG * W, TPI], [1, W]],
                           offset=b * W),
                    in_=xin_v[:, b])
            out_v = out.rearrange("s b (t p) c -> p s b t c", p=P)

            def emit_pair(x_t, s, t):
                # one 512KB chunk per pair keeps the DMA-engine pool free
                # for the latency-critical halo transfers
                nc.sync.dma_start(
                    out=out_v[:, s, :, t],
                    in_=AP(tensor=x_t.ap().tensor,
                           ap=[[XF, P], [W, NIMG], [1, W]],
                           offset=t * PF))

            # state 0 is emitted by the s=0 loop iteration (delayed emits)

            # x0 -> dl16 (fp16, padded layout)
            dl16_data = AP(tensor=dl16.ap().tensor,
                           ap=[[DL, P], [BW, NIMG * TPI], [1, W]], offset=1)
            nc.vector.tensor_copy(
                out=dl16_data,
                in_=xb[0].ap().rearrange("p (g c) -> p g c", g=NIMG * TPI))

            def halo_above_dmas(t):
                # above slots of pair t+1 <- row 127 of pair t (needs G8(t))
                src_a = AP(tensor=dcur.ap().tensor,
                           ap=[[DL, 1], [BW, 2], [1, W]],
                           offset=(P - 1) * DL + blk_off(t, 0))
                for sh in range(2):
                    nc.sync.dma_start(
                        out=AP(tensor=h8[hb].ap().tensor,
                               ap=[[DL, 1], [BW, 2], [1, W]],
                               offset=sh * DL + sh + blk_off(t + 1, 0)),
                        in_=src_a)

            def halo_below_dmas(t):
                # below slots of pair t <- row 0 of pair t+1 (needs G8(t+1))
                src_b = AP(tensor=dcur.ap().tensor,
                           ap=[[DL, 1], [BW, 2], [1, W]],
                           offset=blk_off(t + 1, 0))
                for sh in range(2):
                    nc.gpsimd.dma_start(
                        out=AP(tensor=h8[hb].ap().tensor,
                               ap=[[DL, 1], [BW, 2], [1, W]],
                               offset=(2 + sh) * DL + sh + blk_off(t, 0)),
                        in_=src_b)

            def halo_dmas16(t):
                # init fp16 path: pre-shifted rows into h16i (6 rows, HF)
                for j, dc in ((0, -1), (1, 0), (2, 1)):
                    d0, d1 = (1, W) if dc == -1 else ((0, W) if dc == 0
                                                      else (0, W - 1))
                    s0 = d0 + dc
                    n = d1 - d0
                    dsta = AP(tensor=h16i.ap().tensor,
                              ap=[[HF, 1], [W, 2], [1, n]],
                              offset=j * HF + (t + 1) * PF + d0)
                    srca = AP(tensor=dl16.ap().tensor,
                              ap=[[DL, 1], [BW, 2], [1, n]],
                              offset=(P - 1) * DL + blk_off(t, 0) + s0)
                    nc.sync.dma_start(out=dsta, in_=srca)
                    dstb = AP(tensor=h16i.ap().tensor,
                              ap=[[HF, 1], [W, 2], [1, n]],
                              offset=(3 + j) * HF + t * PF + d0)
                    srcb = AP(tensor=dl16.ap().tensor,
                              ap=[[DL, 1], [BW, 2], [1, n]],
                              offset=blk_off(t + 1, 0) + s0)
                    nc.sync.dma_start(out=dstb, in_=srcb)

            def banded_mms(t, start):
                # 6 fp8 DoubleRow matmuls for pair t (3 per image block)
                cp = cps[t]
                for b in range(NIMG):
                    base = blk_off(t, b)
                    o = b * W
                    # mm1: taps (c-1, c+1) hi: base-1, i-stride 2
                    rhs1 = AP(tensor=dcur.ap().tensor,
                              ap=[[DL, P], [2, 2], [1, W]], offset=base - 1)
                    # mm2: taps (c, c) hi+lo: i-stride 0
                    rhs2 = AP(tensor=dcur.ap().tensor,
                              ap=[[DL, P], [0, 2], [1, W]], offset=base)
                    for wi_, rhs in ((0, rhs1), (1, rhs2)):
                        nc.tensor.matmul(
                            out=cp[:, o:o + W],
                            lhsT=wmm[wi_].ap().rearrange(
                                "p (i m) -> p i m", i=2),
                            rhs=rhs, start=start and wi_ == 0, stop=False,
                            perf_mode=DR)

            def halo_mm(t):
                for b in range(NIMG):
                    base = blk_off(t, b)
                    rhs = AP(tensor=h8[hb].ap().tensor,
                             ap=[[DL, 6], [2, 2], [1, W]], offset=base - 1)
                    nc.tensor.matmul(
                        out=cps[t][:, b * W:(b + 1) * W],
                        lhsT=whm[VAR[t]].ap().rearrange("p (i m) -> p i m",
                                                        i=2),
                        rhs=rhs, start=False, stop=True, perf_mode=DR)

            # ---- init conv: fp16 banded + fp16 halo ----
            for t in range(NPAIR):
                if t < NPAIR - 1:
                    halo_dmas16(t)
            for t in range(NPAIR):
                cp = cps[t]
                for b in range(NIMG):
                    base = blk_off(t, b)
                    o = b * W
                    for j, dc in ((0, -1), (1, 0), (2, 1)):
                        rhs = AP(tensor=dl16.ap().tensor,
                                 ap=[[DL, P], [1, W]], offset=base + dc)
                        nc.tensor.matmul(out=cp[:, o:o + W],
                                         lhsT=wi[j].ap(), rhs=rhs,
                                         start=j == 0,
                                         stop=False)
                # init halo (fp16 plain, 6-row contraction)
                for b in range(NIMG):
                    nc.tensor.matmul(
                        out=cp[:, b * W:(b + 1) * W], lhsT=whi[VAR[t]].ap(),
                        rhs=h16i.ap()[0:6, t * PF + b * W:
                                      t * PF + (b + 1) * W],
                        start=False, stop=True)

            # ---------------- steps ----------------
            for s in range(steps):
                x_cur, x_new = xb[s % 2], xb[(s + 1) % 2]
                last = s == steps - 1
                hb = s % 2
                dcur = dl8[s % 2]

                def g8_stt_emit(t, eb):
                    # G8 = DErf(al*c + bias) -> fp8 into dl8 (padded layout)
                    g_out = AP(tensor=dcur.ap().tensor,
                               ap=[[DL, P], [BW, 2], [1, W]],
                               offset=blk_off(t, 0))
                    nc.scalar.activation(
                        out=g_out, in_=cps[t][:, 0:PF],
                        func=AF.Derivative_Erf,
                        bias=bias.ap()[:, s * 3 + VIDX[VAR[t]]:
                                       s * 3 + VIDX[VAR[t]] + 1],
                        scale=al_)
                    # x_new = q_s * G8 + x_cur
                    g_in = AP(tensor=dcur.ap().tensor,
                              ap=[[DL, P], [BW, 2], [1, W]],
                              offset=blk_off(t, 0))
                    nc.vector.scalar_tensor_tensor(
                        out=x_new.ap()[:, t * PF:(t + 1) * PF],
                        in0=g_in, scalar=q_s,
                        in1=x_cur.ap()[:, t * PF:(t + 1) * PF],
                        op0=OP.mult, op1=OP.add)
                    # emit the PREVIOUS state's pair t (wait long satisfied;
                    # 512KB chunks keep the DMA pool free for halo dmas)
                    eb.dma_start(
                        out=out_v[:, s, :, t],
                        in_=AP(tensor=x_cur.ap().tensor,
                               ap=[[XF, P], [W, NIMG], [1, W]],
                               offset=t * PF))

                # visit order sigma = [1, 0, 2, 3]: the T pair (0) and B
                # pair (3) need only one neighbour each, so their PSUM stops
                # land early and the next step's ACT starts sooner.
                cv = not last
                g8_stt_emit(1, nc.scalar)
                if cv:
                    halo_below_dmas(0)
                    halo_above_dmas(1)
                    banded_mms(1, False)
                g8_stt_emit(0, nc.scalar)
                if cv:
                    halo_above_dmas(0)
                    banded_mms(0, False)
                    halo_mm(0)
                g8_stt_emit(2, nc.gpsimd)
                if cv:
                    halo_below_dmas(1)
                    halo_above_dmas(2)
                    halo_mm(1)
                    banded_mms(2, False)
                g8_stt_emit(3, nc.gpsimd)
                if cv:
                    halo_below_dmas(2)
                    halo_mm(2)
                    banded_mms(3, False)
                    halo_mm(3)
            # tail: emit the final state
            for t in range(NPAIR):
                emit_pair(xb[steps % 2], steps, t)

    nc.compile()
    return nc


# --------------------------------------------------------------------------
# Entry point
# --------------------------------------------------------------------------

def kernel(x, k, w1, b1, w2, steps):
    global LAST_RESULTS
    steps = int(np.asarray(steps))
    x = np.asarray(x, np.float32)
    k = np.asarray(k, np.float32).reshape(3, 3)
    B = x.shape[0]
    assert B == NIMG * NCORES and x.shape[-2:] == (W, W)

    params = _get_params(np.asarray(w1, np.float64),
                         np.asarray(b1, np.float64),
                         np.asarray(w2, np.float64))

    key = (steps, k.tobytes(), tuple(params))
    nc = _NC_CACHE.get(key)
    if nc is None:
        nc = _build_nc(k, params, steps)
        _NC_CACHE.clear()
        _NC_CACHE[key] = nc

    xs = np.ascontiguousarray(x.reshape(B, W, W))
    in_maps = [{"x": np.ascontiguousarray(xs[NIMG * i:NIMG * (i + 1)])}
               for i in range(NCORES)]

    from concourse.bass_utils import run_bass_kernel_spmd
    res = run_bass_kernel_spmd(nc, in_maps, core_ids=list(range(NCORES)))
    LAST_RESULTS = res

    full = np.concatenate([np.asarray(r["out"]) for r in res.results], axis=1)
    full = full.astype(np.float32)
    # add back the s*A drift excluded from the on-chip state
    drift = (np.float64(params[0])
             * np.arange(steps + 1)).astype(np.float32)
    full += drift[:, None, None, None]
    return np.ascontiguousarray(full[:, :, None])


if __name__ == "__main__":
    rng = np.random.default_rng(0)
    x = rng.standard_normal((16, 1, W, W), dtype=np.float32)
    k = rng.standard_normal((1, 1, 3, 3)).astype(np.float32)
    w1 = (rng.standard_normal((10, 1)) * 0.5).astype(np.float32)
    b1 = (rng.standard_normal((10,)) * 0.1).astype(np.float32)
    w2 = (rng.standard_normal((1, 10)) * 0.5).astype(np.float32)
    out = kernel(x=x, k=k, w1=w1, b1=b1, w2=w2, steps=16)
    print("out", out.shape, out.dtype)
